# revision 6
# baseline (speedup 1.0000x reference)
"""Trainium2 Bass kernel for nn_DualBranchDecoder.

Dual-branch band-split decoder: per-band GroupNorm -> fc1(C=128->H=512)+tanh
-> per-band fc2(H->w_k) -> sigmoid mag mask / tanh phase offset -> complex out.

Sharding: data-parallel over batch B=8 across 8 NeuronCores (one sample per
core). All weight preprocessing (transposes, gamma/beta folding, fp32r
rounding) happens on host; the device does stats, normalize, matmuls (fp32r),
activations and the final complex assembly.
"""
import sys
sys.path.insert(0, '/opt/trn_rl_repo')

import numpy as np
import ml_dtypes

import concourse.bacc as bacc
import concourse.tile as tile
import concourse.mybir as mybir
from concourse.bass_utils import run_bass_kernel_spmd

F32 = mybir.dt.float32
F32R = mybir.dt.float32r
BF16 = mybir.dt.bfloat16
FP16 = mybir.dt.float16
H1DT = FP16
W1DT = FP16
W2DT = FP16
FCDT = FP16
AF = mybir.ActivationFunctionType
ALU = mybir.AluOpType

# problem constants (hardcoded per contract)
B, C, T = 8, 128, 512
BANDS = [2] + [3] * 10 + [8] * 12 + [16] * 7 + [17]
K = len(BANDS)                      # 31
F = sum(BANDS)                      # 257
H = 4 * C                           # 512
NHC = H // 128                      # 4 h-chunks
EPS = 1e-5

OFFS = np.concatenate([[0], np.cumsum(BANDS)]).astype(int)   # band start freqs
WPADS = [w + (w & 1) for w in BANDS]                         # fp32r even-M pad
WOFFS = np.concatenate([[0], np.cumsum(WPADS)]).astype(int)
WPTOT = int(WOFFS[-1])

QUADS = [(4 * i, 4) for i in range(7)] + [(28, 3)]
MAGIC = float(1.5 * 2 ** 23)
INV2PI = float(1.0 / (2 * np.pi))
N2PI = float(-2 * np.pi)
PI = float(np.pi)

_cache = {}


def _round_f32r(x):
    hi = x.astype(ml_dtypes.bfloat16).astype(np.float32)
    lo = (x - hi).astype(ml_dtypes.bfloat16).astype(np.float32)
    return (hi + lo).astype(np.float32)


def _prep_branch(gamma, beta, W1, b1, W2, b2):
    """Host-side constant prep for one branch."""
    # W1gT[c, k*H + h] = W1[k,h,c] * gamma[k,c]
    W1g = W1 * gamma[:, None, :]                      # [K, H, C]
    W1gT = np.ascontiguousarray(W1g.transpose(2, 0, 1).reshape(C, K * H))
    W1gT = W1gT.astype(np.float16)
    # b1p[k,h] = b1[k,h] + sum_c W1[k,h,c]*beta[k,c];  row layout [1, K*H]
    b1p = b1 + np.einsum('khc,kc->kh', W1, beta)      # [K, H]
    b1pT = np.zeros((128, K * NHC), np.float32)
    for k in range(K):
        for hc in range(NHC):
            b1pT[:, k * NHC + hc] = b1p[k, hc * 128:(hc + 1) * 128]
    # W2Tp[p, hc*WPTOT + woff_k + j] = W2[off_k + j, hc*128 + p], zero-pad odd
    W2Tp = np.zeros((128, NHC * WPTOT), np.float32)
    for k in range(K):
        w, off, woff = BANDS[k], OFFS[k], WOFFS[k]
        for hc in range(NHC):
            W2Tp[:, hc * WPTOT + woff: hc * WPTOT + woff + w] = \
                W2[off:off + w, hc * 128:(hc + 1) * 128].T
    W2Tp = W2Tp.astype(np.float16)
    # b2g[32*r + p, q] = b2[off_{k0+r} + p] (p < w) for quad q
    b2g = np.zeros((128, len(QUADS)), np.float32)
    for q, (k0, nb) in enumerate(QUADS):
        for r in range(nb):
            k = k0 + r
            b2g[32 * r:32 * r + BANDS[k], q] = b2[OFFS[k]:OFFS[k] + BANDS[k]]
    return W1gT, b1pT, W2Tp, b2g


def _build():
    nc = bacc.Bacc("TRN2", target_bir_lowering=False)

    # per-core inputs
    ins = {}
    for br in ("m", "p"):
        ins[f"feat_{br}"] = nc.dram_tensor(f"feat_{br}", [C, K * T], FP16,
                                           kind="ExternalInput")
        ins[f"w1gt_{br}"] = nc.dram_tensor(f"w1gt_{br}", [C, K * H], W1DT,
                                           kind="ExternalInput")
        ins[f"b1pt_{br}"] = nc.dram_tensor(f"b1pt_{br}", [128, K * NHC], F32,
                                           kind="ExternalInput")
        ins[f"w2tp_{br}"] = nc.dram_tensor(f"w2tp_{br}", [128, NHC * WPTOT], W2DT,
                                           kind="ExternalInput")
        ins[f"b2c_{br}"] = nc.dram_tensor(f"b2c_{br}", [128, len(QUADS)], F32,
                                          kind="ExternalInput")
        ins[f"noisy_{br}"] = nc.dram_tensor(f"noisy_{br}", [F, T], F32,
                                            kind="ExternalInput")
    ones_col_d = nc.dram_tensor("ones_col", [128, 1], F32, kind="ExternalInput")
    ones_row_d = nc.dram_tensor("ones_row", [1, 128], F32, kind="ExternalInput")
    halfpi_d = nc.dram_tensor("halfpi", [128, 1], F32, kind="ExternalInput")
    out_d = nc.dram_tensor("out", [F, 2 * T], F32, kind="ExternalOutput")

    with tile.TileContext(nc) as tc:
        with (
            tc.tile_pool(name="featk", bufs=3) as featk_pool,
            tc.tile_pool(name="w1t", bufs=2) as w1t_pool,
            tc.tile_pool(name="fcent", bufs=4) as fcent_pool,
            tc.tile_pool(name="h1sb", bufs=3) as h1sb_pool,
            tc.tile_pool(name="band", bufs=4) as band_pool,
            tc.tile_pool(name="const", bufs=1) as const_pool,
            tc.tile_pool(name="statsb", bufs=2) as stats_pool,
            tc.tile_pool(name="fin", bufs=2) as fin_pool,
            tc.tile_pool(name="mainps", bufs=1, space="PSUM") as main_ps,
        ):
            # ---- critical-path first: quad-0 mag fetches before anything else ----
            k0_0, nb_0 = QUADS[0]
            fq0 = featk_pool.tile([128, nb_0 * T], FP16, tag="featq", name="featq_m_0")
            nc.sync.dma_start(fq0[:], ins["feat_m"][:, k0_0 * T:(k0_0 + nb_0) * T])
            wq0 = w1t_pool.tile([128, nb_0 * H], W1DT, tag="w1q", name="w1q_m_0")
            nc.sync.dma_start(wq0[:], ins["w1gt_m"][:, k0_0 * H:(k0_0 + nb_0) * H])

            # ---- constants ----
            ones_col = const_pool.tile([128, 1], F32)
            nc.sync.dma_start(ones_col[:], ones_col_d[:])
            ones_row = const_pool.tile([1, 128], F32)
            nc.sync.dma_start(ones_row[:], ones_row_d[:])
            halfpi = const_pool.tile([128, 1], F32)
            nc.sync.dma_start(halfpi[:], halfpi_d[:])

            cb = {}
            for br in ("m", "p"):
                b1pt = const_pool.tile([128, K * NHC], F32, tag=f"b1pt_{br}", name=f"b1pt_{br}")
                nc.sync.dma_start(b1pt[:], ins[f"b1pt_{br}"][:])
                w2tp = const_pool.tile([128, NHC * WPTOT], W2DT, tag=f"w2tp_{br}", name=f"w2tp_{br}")
                nc.sync.dma_start(w2tp[:], ins[f"w2tp_{br}"][:])
                b2c = const_pool.tile([128, len(QUADS)], F32, tag=f"b2c_{br}", name=f"b2c_{br}")
                nc.sync.dma_start(b2c[:], ins[f"b2c_{br}"][:])
                cb[br] = (b1pt, w2tp, b2c)

            # ---- PE warm-up: ~5us of continuous matmul to trip HAM un-throttle ----
            for wi in range(16):
                wps = main_ps.tile([128, T], F32, tag="h1ps", bufs=5,
                                   name=f"warm_{wi}")
                nc.tensor.matmul(wps[:], wq0[:, 0:128], wq0[:, 0:T],
                                 start=True, stop=True)

            # ---- fused per-quad stats + band pipeline ----
            masks = {}
            for br in ("m", "p"):
                masks[br] = const_pool.tile([128, 2 * T], F32, tag=f"mask_{br}", name=f"mask_{br}")
                masks[br + "2"] = const_pool.tile([1, T], F32, tag=f"mask2_{br}", name=f"mask2_{br}")

            for q, (k0, nb) in enumerate(QUADS):
                for br in ("m", "p"):
                    b1pt, w2tp, b2c = cb[br]
                    if q == 0 and br == "m":
                        fq, wq = fq0, wq0
                    else:
                        fq = featk_pool.tile([128, nb * T], FP16, tag="featq",
                                             name=f"featq_{br}_{q}")
                        nc.sync.dma_start(
                            fq[:], ins[f"feat_{br}"][:, k0 * T:(k0 + nb) * T])
                        wq = w1t_pool.tile([128, nb * H], W1DT, tag="w1q",
                                           name=f"w1q_{br}_{q}")
                        nc.sync.dma_start(
                            wq[:], ins[f"w1gt_{br}"][:, k0 * H:(k0 + nb) * H])

                    # quad stats: per-partition bn stats -> cross-partition sums
                    st_q = stats_pool.tile([128, nb * 6], F32, tag="st_q",
                                           name=f"st_{br}_{q}")
                    ag_q = stats_pool.tile([128, nb * 2], F32, tag="ag_q",
                                           name=f"ag_{br}_{q}")
                    for r in range(nb):
                        nc.vector.bn_stats(st_q[:, r * 6:(r + 1) * 6],
                                           fq[:, r * T:(r + 1) * T])
                        nc.vector.bn_aggr(ag_q[:, r * 2:(r + 1) * 2],
                                          st_q[:, r * 6:(r + 1) * 6])
                    ag3 = ag_q[:].rearrange("c (k two) -> c k two", two=2)
                    mean_ap = ag3[:, :, 0]
                    var_ap = ag3[:, :, 1]
                    sums = stats_pool.tile([128, 2 * nb], F32, tag="sums",
                                           name=f"sums_{br}_{q}")
                    nc.vector.tensor_copy(sums[:, 0:nb], mean_ap)
                    tmp = stats_pool.tile([128, nb], F32, tag="tmp",
                                          name=f"tmp_{br}_{q}")
                    nc.vector.tensor_mul(tmp[:], mean_ap, mean_ap)
                    nc.vector.tensor_add(sums[:, nb:2 * nb], tmp[:], var_ap)
                    ps_s = main_ps.tile([1, 2 * nb], F32, tag="ps_s", bufs=1,
                                        name=f"ps_s_{br}_{q}")
                    nc.tensor.matmul(ps_s[:], ones_col[:], sums[:],
                                     start=True, stop=True)
                    g = stats_pool.tile([1, 2 * nb], F32, tag="g",
                                        name=f"g_{br}_{q}")
                    nc.vector.tensor_scalar_mul(g[:], ps_s[:], 1.0 / C)
                    gm2 = stats_pool.tile([1, nb], F32, tag="gm2",
                                          name=f"gm2_{br}_{q}")
                    nc.vector.tensor_mul(gm2[:], g[:, 0:nb], g[:, 0:nb])
                    gvar = stats_pool.tile([1, nb], F32, tag="gvar",
                                           name=f"gvar_{br}_{q}")
                    nc.vector.tensor_sub(gvar[:], g[:, nb:2 * nb], gm2[:])
                    # inv = rsqrt(gvar + EPS), pure-DVE (quake seed + 3 Newton)
                    vv = stats_pool.tile([1, nb], F32, tag="vv",
                                         name=f"vv_{br}_{q}")
                    nc.vector.tensor_scalar_add(vv[:], gvar[:], EPS)
                    I32 = mybir.dt.int32
                    yy = stats_pool.tile([1, nb], F32, tag="yy",
                                         name=f"yy_{br}_{q}")
                    nc.vector.tensor_scalar(yy[:].bitcast(I32), vv[:].bitcast(I32),
                                            1, -1, op0=ALU.arith_shift_right,
                                            op1=ALU.bitwise_xor)
                    nc.vector.tensor_scalar_add(yy[:].bitcast(I32), yy[:].bitcast(I32),
                                                0x5f3759e0)
                    invmean = stats_pool.tile([1, 2 * nb], F32, tag="invmean",
                                              name=f"invmean_{br}_{q}")
                    tnr = stats_pool.tile([1, nb], F32, tag="tnr",
                                          name=f"tnr_{br}_{q}")
                    for it in range(3):
                        nc.vector.tensor_mul(tnr[:], yy[:], yy[:])
                        nc.vector.tensor_mul(tnr[:], tnr[:], vv[:])
                        nc.vector.tensor_scalar(tnr[:], tnr[:], -0.5, 1.5,
                                                op0=ALU.mult, op1=ALU.add)
                        dst = yy[:] if it < 2 else invmean[:, 0:nb]
                        nc.vector.tensor_mul(dst, yy[:], tnr[:])
                    nc.vector.tensor_copy(invmean[:, nb:2 * nb], g[:, 0:nb])
                    ps_b = main_ps.tile([128, 2 * nb], F32, tag="ps_s", bufs=1,
                                        name=f"ps_b_{br}_{q}")
                    nc.tensor.matmul(ps_b[:], ones_row[:], invmean[:],
                                     start=True, stop=True)
                    bbq = stats_pool.tile([128, 2 * nb], F32, tag="bbq", bufs=3,
                                          name=f"bbq_{br}_{q}")
                    nc.vector.tensor_copy(bbq[:], ps_b[:])
                    # bbq[:, r] = inv ; bbq[:, nb+r] = mean

                    h1s = []
                    for r in range(nb):
                        k = k0 + r
                        fcent = fcent_pool.tile([128, T], FCDT)
                        nc.vector.tensor_scalar(fcent[:], fq[:, r * T:(r + 1) * T],
                                                bbq[:, nb + r:nb + r + 1],
                                                bbq[:, r:r + 1],
                                                op0=ALU.subtract, op1=ALU.mult)
                        h1sb = h1sb_pool.tile([128, NHC * T], H1DT, bufs=6)
                        h1s.append(h1sb)
                        for hc in range(NHC):
                            h1ps = main_ps.tile([128, T], F32, tag="h1ps", bufs=5,
                                                name=f"h1ps_{br}_{k}_{hc}")
                            for j in range(4):
                                nc.tensor.matmul(
                                    h1ps[32 * j:32 * j + 32, :],
                                    wq[:, (r * NHC + hc) * 128 + 32 * j:
                                          (r * NHC + hc) * 128 + 32 * j + 32],
                                    fcent[:], start=True, stop=True,
                                    tile_position=(0, 32 * j))
                            nc.scalar.activation(
                                h1sb[:, hc * T:(hc + 1) * T], h1ps[:],
                                AF.Tanh, bias=b1pt[:, k * NHC + hc:k * NHC + hc + 1])
                    # quad fc2: hc-outer so the 4 bands' col-strips overlap in PE
                    fc2g = main_ps.tile([128, T], F32, tag="fc2ps", bufs=2,
                                        name=f"fc2g_{br}_{q}")
                    for hc in range(NHC):
                        for r in range(nb):
                            k = k0 + r
                            wp, woff = WPADS[k], int(WOFFS[k])
                            nc.tensor.matmul(
                                fc2g[32 * r:32 * r + wp, :],
                                w2tp[:, hc * WPTOT + woff: hc * WPTOT + woff + wp],
                                h1s[r][:, hc * T:(hc + 1) * T],
                                start=(hc == 0), stop=(hc == NHC - 1),
                                tile_position=(0, 32 * r))
                    grp_t = band_pool.tile([128, T], F32, tag="band")
                    nc.scalar.activation(grp_t[:], fc2g[:],
                                         AF.Sigmoid if br == "m" else AF.Tanh,
                                         bias=b2c[:, q:q + 1])
                    dma_eng = nc.sync if q == len(QUADS) - 1 else nc.gpsimd
                    for r in range(nb):
                        k = k0 + r
                        w, off = BANDS[k], int(OFFS[k])
                        j0, r0 = off // 128, off % 128
                        if off + w <= (j0 + 1) * 128:
                            dma_eng.dma_start(
                                masks[br][r0:r0 + w, j0 * T:(j0 + 1) * T],
                                grp_t[32 * r:32 * r + w, :])
                        else:
                            n1 = (j0 + 1) * 128 - off
                            dma_eng.dma_start(
                                masks[br][r0:128, j0 * T:(j0 + 1) * T],
                                grp_t[32 * r:32 * r + n1, :])
                            rem = w - n1
                            if j0 + 1 < 2:
                                dma_eng.dma_start(
                                    masks[br][0:rem, (j0 + 1) * T:(j0 + 2) * T],
                                    grp_t[32 * r + n1:32 * r + w, :])
                            else:
                                dma_eng.dma_start(masks[br + "2"][0:rem, :],
                                                  grp_t[32 * r + n1:32 * r + w, :])

            # ---- final complex assembly per f-chunk ----
            for j in range(3):
                rows = 128 if j < 2 else 1
                if j < 2:
                    mask_ap = masks["m"][:, j * T:(j + 1) * T]
                    poff_ap = masks["p"][:, j * T:(j + 1) * T]
                else:
                    mask_ap = masks["m2"][0:1, :]
                    poff_ap = masks["p2"][0:1, :]
                nmag = fin_pool.tile([rows, T], F32, tag="nmag")
                nc.gpsimd.dma_start(nmag[:], ins["noisy_m"][j * 128:j * 128 + rows, :])
                nph = fin_pool.tile([rows, T], F32, tag="nph")
                nc.gpsimd.dma_start(nph[:], ins["noisy_p"][j * 128:j * 128 + rows, :])

                ang = fin_pool.tile([rows, T], F32, tag="ang")
                nc.vector.scalar_tensor_tensor(ang[:], poff_ap, PI, nph[:],
                                               op0=ALU.mult, op1=ALU.add)
                enh = fin_pool.tile([rows, T], F32, tag="enh")
                nc.vector.tensor_mul(enh[:], mask_ap, nmag[:])
                # sin: n = round(ang/2pi) via magic; ws = ang - 2pi*n
                t2 = fin_pool.tile([rows, T], F32, tag="t2")
                nc.vector.tensor_scalar(t2[:], ang[:], INV2PI, MAGIC,
                                        op0=ALU.mult, op1=ALU.add)
                m2pin = fin_pool.tile([rows, T], F32, tag="m2pin")
                nc.vector.tensor_scalar(m2pin[:], t2[:], MAGIC, N2PI,
                                        op0=ALU.subtract, op1=ALU.mult)
                nc.vector.tensor_add(m2pin[:], ang[:], m2pin[:])
                sn = fin_pool.tile([rows, T], F32, tag="sn")
                nc.scalar.activation(sn[:], m2pin[:], AF.Sin)
                # cos: n' = round((ang/2pi) + 0.25); wc = ang - 2pi*n'; Sin(wc + pi/2)
                t2c = fin_pool.tile([rows, T], F32, tag="t2c")
                nc.vector.tensor_scalar(t2c[:], ang[:], INV2PI, 0.25,
                                        op0=ALU.mult, op1=ALU.add)
                nc.vector.tensor_scalar_add(t2c[:], t2c[:], MAGIC)
                m2pinc = fin_pool.tile([rows, T], F32, tag="m2pinc")
                nc.vector.tensor_scalar(m2pinc[:], t2c[:], MAGIC, N2PI,
                                        op0=ALU.subtract, op1=ALU.mult)
                nc.vector.tensor_add(m2pinc[:], ang[:], m2pinc[:])
                cn = fin_pool.tile([rows, T], F32, tag="cn")
                nc.scalar.activation(cn[:], m2pinc[:], AF.Sin, bias=halfpi[0:rows, :])

                ot = fin_pool.tile([rows, 2 * T], F32, tag="ot")
                ot2 = ot[:].rearrange("p (t two) -> p t two", two=2)
                nc.vector.tensor_mul(ot2[:, :, 0], enh[:], cn[:])
                nc.vector.tensor_mul(ot2[:, :, 1], enh[:], sn[:])
                nc.sync.dma_start(out_d[j * 128:j * 128 + rows, :], ot[:])

    nc.compile()
    return nc


def kernel(mag_features, phase_features, noisy_mag, noisy_phase,
           mag_gamma, mag_beta, mag_W1, mag_b1, mag_W2, mag_b2,
           ph_gamma, ph_beta, ph_W1, ph_b1, ph_W2, ph_b2):
    if "nc" not in _cache:
        _cache["nc"] = _build()
    nc = _cache["nc"]

    mW1gT, mb1pT, mW2Tp, mb2c = _prep_branch(
        np.asarray(mag_gamma), np.asarray(mag_beta), np.asarray(mag_W1),
        np.asarray(mag_b1), np.asarray(mag_W2), np.asarray(mag_b2))
    pW1gT, pb1pT, pW2Tp, pb2c = _prep_branch(
        np.asarray(ph_gamma), np.asarray(ph_beta), np.asarray(ph_W1),
        np.asarray(ph_b1), np.asarray(ph_W2), np.asarray(ph_b2))

    shared = dict(
        w1gt_m=mW1gT, b1pt_m=mb1pT, w2tp_m=mW2Tp, b2c_m=mb2c,
        w1gt_p=pW1gT, b1pt_p=pb1pT, w2tp_p=pW2Tp, b2c_p=pb2c,
        ones_col=np.ones((128, 1), np.float32),
        ones_row=np.ones((1, 128), np.float32),
        halfpi=np.full((128, 1), np.pi / 2, np.float32),
    )
    mag_features = np.asarray(mag_features)
    phase_features = np.asarray(phase_features)
    noisy_mag = np.asarray(noisy_mag)
    noisy_phase = np.asarray(noisy_phase)

    in_maps = []
    for b in range(B):
        m = dict(shared)
        # [C, T, K] -> [C, K, T] k-major, contiguous per-band slices
        m["feat_m"] = np.ascontiguousarray(
            mag_features[b].transpose(0, 2, 1)).reshape(C, K * T).astype(np.float16)
        m["feat_p"] = np.ascontiguousarray(
            phase_features[b].transpose(0, 2, 1)).reshape(C, K * T).astype(np.float16)
        m["noisy_m"] = np.ascontiguousarray(noisy_mag[b])
        m["noisy_p"] = np.ascontiguousarray(noisy_phase[b])
        in_maps.append(m)

    import os
    trace = bool(os.environ.get("BASS_PROFILE"))
    res = run_bass_kernel_spmd(nc, in_maps, list(range(B)), trace=trace)
    _cache["last_result"] = res
    out = np.stack([res.results[b]["out"].view(np.complex64) for b in range(B)])
    return out

